# revision 38
# baseline (speedup 1.0000x reference)
"""MoE gate routing kernel for Trainium2 (8 NeuronCores, data-parallel over tokens).

Computes, for x[8192,7168], weight[256,7168], bias[256]:
    scores = sigmoid(x @ weight.T + bias)            # [N, 256]
    group top-2 sums over 8 groups of 32 -> pick best group
    top-8 experts within best group (global indices), weights = renormalized
    sigmoid scores * 2.5
Returns (w [8192,8] f32, idx [8192,8] i32).

Strategy: shard tokens 8-way (1024/core). Host pre-packs x and weight into the
exact SBUF tile layouts so every DMA descriptor line is a long contiguous run
(28KB/partition vs 1KB for a naive transposed layout). Weights load as 4
K-groups so the first matmuls only wait on ~5.5MB of DMA instead of ~15MB.
Outputs accumulate in SBUF and leave as one DMA per tensor at the end, keeping
the SP DMA stream free of mid-loop waits on the vector pipeline. Matmul runs
as float32r (full-rate fp32). Bias is preloaded into PSUM via a K=1
ones-matmul. Sigmoid on ScalarE; group-top2 / top-8 / renorm on VectorE via
tensor_reduce, match_replace, max/max_index.
"""

import sys

sys.path.insert(0, "/opt/trn_rl_repo")

from concurrent.futures import ThreadPoolExecutor

import numpy as np

import concourse.bass as bass
from concourse import bacc
import concourse.mybir as mybir
from concourse.bass_utils import run_bass_kernel_spmd
from concourse.tile import TileContext

N_CORES = 8
N_TOK = 8192
TOK_PC = N_TOK // N_CORES  # 1024 tokens per core
D = 7168
E = 256
G = 8  # groups
EPG = E // G  # 32 experts per group
TOPK = 8
ROUTE_SCALE = 2.5
KC = D // 128  # 56 k-chunks
XBUF_T = 128  # tokens per x DMA buffer
NBUF = TOK_PC // XBUF_T  # 4 buffers
SUB = XBUF_T // 128  # 2 subtiles per buffer
NTILE = TOK_PC // 128  # 8 token tiles
KCH = KC // 2  # 28 k-chunks per DMA half
NQ = 4  # weight K-groups
KCQ = KC // NQ  # 14 k-chunks per group

f32 = mybir.dt.float32
f32r = mybir.dt.float32r
i32 = mybir.dt.int32
u32 = mybir.dt.uint32
AX = mybir.AxisListType
OP = mybir.AluOpType
ACTF = mybir.ActivationFunctionType

_cache = {}
LAST_RESULTS = None


def _build():
    nc = bacc.Bacc(None, target_bir_lowering=False)

    # host-packed: x5[tb, p, c, n] = x[tok0 + tb*XBUF_T + n, c*128 + p]
    # full-buffer DMA: 28.7KB contiguous per partition; a c-slice quarter DMA
    # still gets one contiguous 7.2KB run per partition.
    x5 = nc.declare_dram_parameter("x5", [NBUF, 128, KC, XBUF_T], f32, isOutput=False)
    # w5[q, p, c, e] = weight[e, (q*KCQ+c)*128 + p]
    w5 = nc.declare_dram_parameter("w5", [NQ, 128, KCQ, E], f32, isOutput=False)
    bias = nc.declare_dram_parameter("bias", [1, E], f32, isOutput=False)
    # outputs tile-major: w_out[i, p, k] = w for token i*128+p
    w_out = nc.declare_dram_parameter("w_out", [NTILE, 128, TOPK], f32, isOutput=True)
    idx_out = nc.declare_dram_parameter(
        "idx_out", [NTILE, 128, TOPK], i32, isOutput=True
    )

    with TileContext(nc) as tc:
        with (
            tc.tile_pool(name="const", bufs=1) as cpool,
            tc.tile_pool(name="xbuf", bufs=3) as xpool,
            tc.tile_pool(name="sb", bufs=2) as spool,
            tc.tile_pool(name="small", bufs=3) as mpool,
            tc.tile_pool(name="out", bufs=1) as opool,
            tc.tile_pool(name="psum", bufs=6, space="PSUM") as ppool,
        ):
            bias_sb = cpool.tile([1, E], f32)
            nc.sync.dma_start(out=bias_sb, in_=bias[:, :])
            ones_sb = cpool.tile([1, 128], f32)
            nc.vector.memset(ones_sb, 1.0)

            # weight K-groups: separate tiles so early matmuls gate per-group
            wt = [
                cpool.tile([128, KCQ, E], f32r, name=f"wt{q}") for q in range(NQ)
            ]
            for q in range(NQ):
                nc.sync.dma_start(out=wt[q], in_=w5[q].bitcast(f32r))

            # output accumulation tiles (free-dim slice per token tile)
            w_acc = opool.tile([128, NTILE, TOPK], f32)
            i_acc = opool.tile([128, NTILE, TOPK], u32)

            # PE warm-up: the HAM clock gate holds the PE at half rate until
            # ~3.4us of sustained activity, and re-throttles after ~3.4us
            # idle. Full-K dummy matmuls (the monitor tracks whole-array
            # activity; K=1 ones don't register), batched behind the wt0/wt1
            # DMA arrivals, keep the PE busy through the x0 load so tile 0
            # (and via the WAR-gated DMA pipeline, the whole stream) runs at
            # full clock.
            warm_ps = ppool.tile([128, E], f32, tag="warm", bufs=1)
            warm_lhs = cpool.tile([128, 128], f32r)
            nc.vector.memset(warm_lhs.bitcast(f32), 0.0)

            def warm(rhs, n):
                for _ in range(n):
                    nc.tensor.matmul(
                        out=warm_ps[:, : rhs.shape[-1]],
                        lhsT=warm_lhs,
                        rhs=rhs,
                        start=True,
                        stop=True,
                    )

            warm(warm_lhs, 24)
            for q in range(NQ):
                warm(wt[q][:, 0, :], 32)

            for tb in range(NBUF):
                # each buffer is DMA'd as two 28-chunk halves (28.7KB
                # contiguous lines, the best-bandwidth shape): the PE's idle
                # wait at each arrival stays well under the ~3.4us HAM
                # re-throttle window, so matmuls run at full clock for the
                # whole stream.
                xq = []
                for q in range(2):
                    xqt = xpool.tile(
                        [128, KCH, XBUF_T], f32r, tag=f"xh{q}", bufs=3
                    )
                    nc.sync.dma_start(
                        out=xqt,
                        in_=x5[tb, :, q * KCH : (q + 1) * KCH, :].bitcast(f32r),
                    )
                    xq.append(xqt)

                pss = []
                for s in range(SUB):
                    ps = ppool.tile([128, E], f32, tag=f"ps{s}", bufs=3)
                    # bias preload: ps[t, e] = 1 * bias[e]
                    nc.tensor.matmul(
                        out=ps, lhsT=ones_sb, rhs=bias_sb, start=True, stop=False
                    )
                    pss.append(ps)
                # per half: both subtiles' chunk groups (accumulation order
                # within each PSUM group stays bias, c0..c55), then a PE pad
                # so the wait for the next half stays under the HAM window
                for h in range(2):
                    for s in range(SUB):
                        for ch in range(KCH):
                            c = h * KCH + ch
                            nc.tensor.matmul(
                                out=pss[s],
                                lhsT=xq[h][:, ch, s * 128 : (s + 1) * 128],
                                rhs=wt[c // KCQ][:, c % KCQ, :],
                                start=False,
                                stop=(c == KC - 1),
                            )
                    warm(wt[0][:, 0, :], 6)

                for s in range(SUB):
                    ti = tb * SUB + s
                    sig = spool.tile([128, G, EPG], f32, tag="sig")
                    nc.scalar.activation(
                        out=sig.rearrange("p g e -> p (g e)"),
                        in_=pss[s],
                        func=ACTF.Sigmoid,
                    )
                    sig_flat = sig.rearrange("p g e -> p (g e)")

                    # group top-2 sum
                    m1 = mpool.tile([128, G], f32, tag="m1")
                    nc.vector.tensor_reduce(out=m1, in_=sig, axis=AX.X, op=OP.max)
                    scr = spool.tile([128, G, EPG], f32, tag="scr")
                    nc.vector.match_replace(
                        out=scr.rearrange("p g e -> p (g e)"),
                        in_to_replace=m1,
                        in_values=sig_flat,
                        imm_value=-1e30,
                    )
                    gs = mpool.tile([128, G], f32, tag="gs")
                    nc.vector.tensor_reduce(out=gs, in_=scr, axis=AX.X, op=OP.max)
                    nc.vector.tensor_add(gs, gs, m1)  # m1 + m2

                    # one-hot of best group -> multiplicative mask
                    gmax = mpool.tile([128, 1], f32, tag="gmax")
                    nc.vector.tensor_reduce(out=gmax, in_=gs, axis=AX.X, op=OP.max)
                    eq = mpool.tile([128, G, 1], f32, tag="eq")
                    nc.vector.tensor_scalar(
                        eq.rearrange("p g 1 -> p g"), gs, gmax, None, op0=OP.is_ge
                    )
                    # masked scores: kept group unchanged (x1.0), others ->
                    # 0.0, via one stride-0-broadcast multiply over all groups
                    masked = spool.tile([128, G, EPG], f32, tag="masked")
                    sig_b, eq_b = bass.broadcast_tensor_aps(
                        sig[:, :, :], eq[:, :, :]
                    )
                    nc.vector.tensor_tensor(
                        out=masked[:, :, :], in0=sig_b, in1=eq_b, op=OP.mult
                    )
                    masked_flat = masked.rearrange("p g e -> p (g e)")

                    vals8 = mpool.tile([128, TOPK], f32, tag="vals8")
                    nc.vector.max(out=vals8, in_=masked_flat)
                    nc.vector.max_index(
                        out=i_acc[:, ti, :], in_max=vals8, in_values=masked_flat
                    )

                    ssum = mpool.tile([128, 1], f32, tag="ssum")
                    nc.vector.tensor_reduce(
                        out=ssum, in_=vals8, axis=AX.X, op=OP.add
                    )
                    rcp = mpool.tile([128, 1], f32, tag="rcp")
                    nc.vector.reciprocal(out=rcp, in_=ssum)
                    nc.vector.tensor_scalar(
                        w_acc[:, ti, :],
                        vals8,
                        rcp,
                        ROUTE_SCALE,
                        op0=OP.mult,
                        op1=OP.mult,
                    )

            nc.sync.dma_start(out=w_out.rearrange("i p k -> p i k"), in_=w_acc)
            nc.sync.dma_start(
                out=idx_out.rearrange("i p k -> p i k"), in_=i_acc.bitcast(i32)
            )
    nc.compile()
    return nc


def _pack_x(x, c):
    xs = x[c * TOK_PC : (c + 1) * TOK_PC]  # [1024, 7168]
    # x5[tb, p, ck, n] = xs[tb*XBUF_T + n, ck*128 + p]
    v = xs.reshape(NBUF, XBUF_T, KC, 128)  # [tb, n, ck, p]
    return np.ascontiguousarray(v.transpose(0, 3, 2, 1))


def kernel(x, weight, bias):
    global LAST_RESULTS
    x = np.ascontiguousarray(x, dtype=np.float32)
    weight = np.ascontiguousarray(weight, dtype=np.float32)
    bias = np.ascontiguousarray(bias, dtype=np.float32).reshape(1, E)

    if "nc" not in _cache:
        _cache["nc"] = _build()
    nc = _cache["nc"]

    # w5[q, p, cq, e] = weight[e, (q*KCQ+cq)*128 + p]
    w5 = np.ascontiguousarray(
        weight.reshape(E, NQ, KCQ, 128).transpose(1, 3, 2, 0)
    )

    with ThreadPoolExecutor(N_CORES) as ex:
        x_shards = list(ex.map(lambda c: _pack_x(x, c), range(N_CORES)))

    in_maps = [
        {"x5": x_shards[c], "w5": w5, "bias": bias} for c in range(N_CORES)
    ]
    LAST_RESULTS = run_bass_kernel_spmd(nc, in_maps, list(range(N_CORES)))
    res = LAST_RESULTS.results
    w = np.concatenate(
        [res[c]["w_out"].reshape(TOK_PC, TOPK) for c in range(N_CORES)], axis=0
    )
    idx = np.concatenate(
        [res[c]["idx_out"].reshape(TOK_PC, TOPK) for c in range(N_CORES)], axis=0
    )
    return w, idx.astype(np.int32)
